# revision 13
# baseline (speedup 1.0000x reference)
"""Trainium2 Bass kernel for nn_Conv3DNorm (modulated conv3d + demod + lrelu + clamp).

Reference math (styles == ones):
    dcoef[cout] = rsqrt(sum_{cin,kd,kh,kw} weight^2 + 1e-8)
    y = conv3d(x, weight * dcoef, pad=1)            # per-sample, stride 1
    y = leaky_relu(y + bias, 0.2) * sqrt(2)
    y = clip(y, -256, 256)

Sharding: data-parallel over batch. Core i processes sample i (B=8 == n_cores).
Weight/bias replicated. Everything on device except input layout prep:
  - weight pre-transposed on host to [cin, tap, cout] (matmul lhsT layout)
  - conv is computed as 27 accumulated matmuls (one per kernel tap) over a
    zero-padded (H,W)-padded input volume resident in SBUF; depth taps that
    fall outside the volume are skipped (implicit D padding).
  - matmul runs in float32r (TF32-like, 1 cycle/row at N>=512) by default.
"""

import os
import sys

for _p in (
    "/root/.axon_site",
    "/root/.axon_site/_ro/trn_rl_repo",
    "/root/.axon_site/_ro/pypackages",
):
    if os.path.isdir(_p) and _p not in sys.path:
        sys.path.insert(0, _p)

import numpy as np

import concourse.bass as bass  # noqa: F401
import concourse.mybir as mybir
import concourse.tile as tile
from concourse import bacc
from concourse.bass_utils import run_bass_kernel_spmd

# Problem constants (hardcoded per contract).
B = 8
CIN = 128
COUT = 128
D = H = W = 32
K = 3
NTAPS = K * K * K  # 27
HP = H + 2  # 34
WP = W + 2  # 34
NCHUNK = 64  # output chunks of 512 spatial positions: (d, half-of-H)
EPS = 1e-8
S1 = float(np.sqrt(2.0))  # ACT_GAIN * GAIN
CLAMP = 256.0
ALPHA = 0.2

# matmul dtype: "f32r" (TF32-like), "bf16", or "f32" (exact, 4x slower)
MM_MODE = os.environ.get("CONV_MM_MODE", "f32r")

LAST_RESULTS = None  # BassKernelResults of the most recent run (for test.py)

_CACHED = {}


def _build_nc(mode: str):
    dt = mybir.dt
    # x / w live in the matmul dtype end-to-end (f32r is a bit-identical
    # reinterpretation of fp32 that the PE runs at 1 cycle/row).
    io_dt = {"f32r": dt.float32r, "bf16": dt.bfloat16, "f32": dt.float32}[mode]

    nc = bacc.Bacc("TRN2")
    x_d = nc.dram_tensor("x", [CIN, D, H, W], io_dt, kind="ExternalInput")
    w_d = nc.dram_tensor("w", [CIN, NTAPS, COUT], io_dt, kind="ExternalInput")
    b_d = nc.dram_tensor("bias", [COUT, 1], dt.float32, kind="ExternalInput")
    y_d = nc.dram_tensor("y", [COUT, NCHUNK, 512], dt.float32, kind="ExternalOutput")

    def asf32(ap):
        return ap.bitcast(dt.float32) if mode == "f32r" else ap

    with tile.TileContext(nc) as tc:
        with (
            tc.tile_pool(name="big", bufs=1) as big,
            tc.tile_pool(name="small", bufs=1) as small,
            tc.tile_pool(name="sq", bufs=2) as sqp,
            tc.tile_pool(name="epiv", bufs=4) as vp,
            tc.tile_pool(name="epio", bufs=4) as op,
        ):
            # ---- weights + bias in SBUF ----
            w_sb = big.tile([CIN, NTAPS, COUT], io_dt)
            nc.sync.dma_start(w_sb[:], w_d[:])
            bias_sb = small.tile([COUT, 1], dt.float32)
            nc.sync.dma_start(bias_sb[:], b_d[:])

            # ---- demodulation coefficients on device ----
            # sums[cout] = sum_{cin,tap} w^2  via 27 accumulated matmuls with ones
            ones = small.tile([CIN, 1], dt.float32)
            nc.vector.memset(ones[:], 1.0)
            eps_t = small.tile([COUT, 1], dt.float32)
            nc.vector.memset(eps_t[:], EPS)
            with tc.tile_pool(name="dcps", bufs=1, space="PSUM") as dcps:
                ps_dc = dcps.tile([COUT, 1], dt.float32)
                for t in range(NTAPS):
                    sq = sqp.tile([CIN, COUT], dt.float32)
                    nc.vector.tensor_mul(
                        sq[:], asf32(w_sb[:, t, :]), asf32(w_sb[:, t, :])
                    )
                    nc.tensor.matmul(
                        ps_dc[:], sq[:], ones[:],
                        start=(t == 0), stop=(t == NTAPS - 1),
                    )
                # dscale = sqrt(2) / sqrt(sums + eps); bias_s1 = sqrt(2) * bias
                rsq = small.tile([COUT, 1], dt.float32)
                nc.scalar.activation(
                    rsq[:], ps_dc[:], mybir.ActivationFunctionType.Sqrt, bias=eps_t[:]
                )
            rec = small.tile([COUT, 1], dt.float32)
            nc.vector.reciprocal(rec[:], rsq[:])
            # epilogue computes v = relu(4*a2) + a2 with a2 = 0.2*sqrt2*(psum*dcoef+bias)
            # == sqrt2 * leaky_relu(psum*dcoef + bias, 0.2)
            dscale2 = small.tile([COUT, 1], dt.float32)
            nc.scalar.mul(dscale2[:], rec[:], ALPHA * S1)
            bias2 = small.tile([COUT, 1], dt.float32)
            nc.scalar.mul(bias2[:], bias_sb[:], ALPHA * S1)

            # ---- padded input volume in SBUF: [cin, d, h+2, w+2] ----
            xpad = big.tile([CIN, D, HP, WP], io_dt)
            # zero the (H,W) halo once (bitcast: memset lacks f32r support)
            nc.vector.memset(asf32(xpad[:, :, 0, :]), 0.0)
            nc.vector.memset(asf32(xpad[:, :, HP - 1, :]), 0.0)
            nc.vector.memset(asf32(xpad[:, :, 1 : HP - 1, 0]), 0.0)
            nc.vector.memset(asf32(xpad[:, :, 1 : HP - 1, WP - 1]), 0.0)
            # interior, one DMA per depth slice
            for d in range(D):
                nc.sync.dma_start(
                    xpad[:, d, 1 : HP - 1, 1 : WP - 1], x_d[:, d, :, :]
                )

            # ---- main conv loop ----
            # chunk c -> (d = c//2, h0 = (c%2)*16); groups of 8 chunks share the
            # 8 PSUM banks; taps iterated outer within a group so consecutive
            # matmuls share the same stationary weights.
            with tc.tile_pool(name="ps", bufs=8, space="PSUM") as psp:
                for g in range(NCHUNK // 8):
                    group = [(c // 2, (c % 2) * 16) for c in range(8 * g, 8 * g + 8)]
                    pst = [
                        psp.tile([COUT, 512], dt.float32, name=f"ps_{g}_{i}", tag="ps")
                        for i in range(len(group))
                    ]
                    # per-chunk first/last valid tap (depth taps may be skipped)
                    bounds = []
                    for d, _h0 in group:
                        valid = [
                            t
                            for t in range(NTAPS)
                            if 0 <= d + t // 9 - 1 < D
                        ]
                        bounds.append((valid[0], valid[-1]))
                    for t in range(NTAPS):
                        kd, kh, kw = t // 9, (t // 3) % 3, t % 3
                        for ci, (d, h0) in enumerate(group):
                            di = d + kd - 1
                            if di < 0 or di >= D:
                                continue
                            rhs = xpad[:, di, h0 + kh : h0 + kh + 16, kw : kw + 32]
                            nc.tensor.matmul(
                                pst[ci][:],
                                w_sb[:, t, :],
                                rhs,
                                start=(t == bounds[ci][0]),
                                stop=(t == bounds[ci][1]),
                            )
                    # epilogue: sqrt2*lrelu(psum*dcoef + bias, 0.2) then clamp
                    for ci, (d, h0) in enumerate(group):
                        c = 8 * g + ci
                        a2 = vp.tile([COUT, 512], dt.float32)
                        nc.vector.tensor_scalar(
                            out=a2[:],
                            in0=pst[ci][:],
                            scalar1=dscale2[:],
                            scalar2=bias2[:],
                            op0=mybir.AluOpType.mult,
                            op1=mybir.AluOpType.add,
                        )
                        r1 = vp.tile([COUT, 512], dt.float32, name=f"r1_{c}", tag="r1")
                        nc.scalar.activation(
                            r1[:],
                            a2[:],
                            mybir.ActivationFunctionType.Relu,
                            scale=1.0 / ALPHA - 1.0,
                        )
                        o = op.tile([COUT, 512], dt.float32)
                        nc.vector.scalar_tensor_tensor(
                            out=o[:],
                            in0=r1[:],
                            scalar=1.0,
                            in1=a2[:],
                            op0=mybir.AluOpType.mult,
                            op1=mybir.AluOpType.add,
                        )
                        oc = op.tile([COUT, 512], dt.float32, name=f"oc_{c}", tag="oc")
                        nc.vector.tensor_scalar(
                            out=oc[:],
                            in0=o[:],
                            scalar1=-CLAMP,
                            scalar2=CLAMP,
                            op0=mybir.AluOpType.max,
                            op1=mybir.AluOpType.min,
                        )
                        nc.sync.dma_start(y_d[:, c, :], oc[:])
    nc.compile()
    return nc


def _get_nc(mode: str):
    if mode not in _CACHED:
        _CACHED[mode] = _build_nc(mode)
    return _CACHED[mode]


def kernel(x: np.ndarray, weight: np.ndarray, bias: np.ndarray) -> np.ndarray:
    global LAST_RESULTS
    mode = MM_MODE
    if mode == "bf16":
        import ml_dtypes

        io = ml_dtypes.bfloat16
    else:
        io = np.float32

    x = np.asarray(x)
    weight = np.asarray(weight, dtype=np.float32)
    bias = np.asarray(bias, dtype=np.float32)

    # [cout, cin, kd, kh, kw] -> [cin, (kd kh kw), cout]
    w_prep = np.ascontiguousarray(
        weight.transpose(1, 2, 3, 4, 0).reshape(CIN, NTAPS, COUT).astype(io)
    )
    b_prep = np.ascontiguousarray(bias.reshape(COUT, 1))

    in_maps = [
        {
            "x": np.ascontiguousarray(x[i].astype(io)),
            "w": w_prep,
            "bias": b_prep,
        }
        for i in range(B)
    ]

    nc = _get_nc(mode)
    trace = bool(int(os.environ.get("CONV_TRACE", "0")))
    res = run_bass_kernel_spmd(
        nc,
        in_maps,
        core_ids=list(range(B)),
        trace=trace,
    )
    LAST_RESULTS = res
    out = np.stack(
        [r["y"].reshape(COUT, D, H, W) for r in res.results], axis=0
    ).astype(np.float32)
    return out
